# revision 41
# baseline (speedup 1.0000x reference)
"""Entity-resolution head on 8 TRN2 NeuronCores.

Pure data-parallel: batch dim (256) split 32/core; MLP weights replicated.

Per-core kernel strategy (v2, bf16):
 - Host pregathers the span rows (<=15/span) and the pron/first/last rows,
   casts everything to bf16, and pre-transposes the single-row features so
   the device does no indirect DMA and no transposes for them.
 - Mean span features via mask-stationary matmuls (mask [rows,32] bf16
   stationary, gathered rows [rows,1024] bf16 moving) -> psum [64,1024].
 - MLP matmuls keep activations transposed (features-on-partitions) as the
   stationary operand, weights stream as bf16 moving operand at N=1024.
 - Weights arrive as a few multi-MiB DMAs on the two HWDGE queues
   (sync + scalar), ordered so each layer's weights land just in time.
 - Both branches' layer-1 outputs share one [64,1024] psum so LayerNorm,
   leaky-relu and the affine run once on 64 partitions.
"""

import numpy as np
from ml_dtypes import bfloat16, float8_e4m3

import concourse.bass as bass
import concourse.mybir as mybir
import concourse.tile as tile
from concourse.bass_utils import run_bass_kernel_spmd
from concourse.masks import make_identity
from concourse.tile import add_dep_helper

B, S, H = 256, 512, 1024
HH, LH, NOUT = 512, 512, 3
EPS = 1e-5
NCORES = 8
BC = B // NCORES          # 32 batches per core
LSPAN = 15                # max span length (reference: 1..15)
KROWS = BC * LSPAN        # 480 gathered rows per span side
KPAD = 512                # padded to 4 chunks of 128
NKC = KPAD // 128         # 4 row chunks per side
NWE1C = 6                 # We1 DMA chunks
KT_PER_C = 48 // NWE1C    # k-tiles per We1 chunk

F32 = mybir.dt.float32
BF16 = mybir.dt.bfloat16
FP8 = mybir.dt.float8e4
WE1_SCALE = 4096.0        # We1 quantized to fp8 at x4096; LN absorbs the
                          # scale (be1 is pre-scaled to match)


def _build_program():
    nc = bass.Bass()

    # ---- DRAM parameters (all host-prepped layouts) --------------------
    ga = nc.declare_dram_parameter("ga", [128, NKC, H], BF16, isOutput=False)
    gb = nc.declare_dram_parameter("gb", [128, NKC, H], BF16, isOutput=False)
    mk = nc.declare_dram_parameter("mk", [128, NKC, 2 * BC], BF16, isOutput=False)
    flt = nc.declare_dram_parameter("flt", [128, 8, 5 * BC], BF16, isOutput=False)
    p64 = nc.declare_dram_parameter("p64", [2 * BC, H], F32, isOutput=False)
    gbt = nc.declare_dram_parameter("gbt", [128, 8, 2, 2 * BC], BF16,
                                    isOutput=False)
    p32 = nc.declare_dram_parameter("p32", [BC, 2 * HH + LH + NOUT], F32,
                                    isOutput=False)
    wp1 = nc.declare_dram_parameter("wp1", [128, 8, H], BF16, isOutput=False)
    we1c = [nc.declare_dram_parameter(f"we1c{c}", [128, KT_PER_C, H], FP8,
                                      isOutput=False) for c in range(NWE1C)]
    w2 = nc.declare_dram_parameter("w2", [128, 24, HH], BF16, isOutput=False)
    wc = nc.declare_dram_parameter("wc", [128, 4, 4], BF16, isOutput=False)
    out = nc.declare_dram_parameter("out", [BC, NOUT], F32, isOutput=True)

    with tile.TileContext(nc) as tc:
        with (
            tc.tile_pool(name="singles", bufs=1) as singles,
            tc.tile_pool(name="ps", bufs=4, space="PSUM") as psp,
        ):
            # ---- small constants -----------------------------------------
            ident = singles.tile([64, 64], BF16, tag="ident")
            make_identity(nc, ident[:])
            identf = singles.tile([64, 64], F32, tag="identf")
            nc.gpsimd.memset(identf[:], 0.0)
            nc.gpsimd.affine_select(
                out=identf[:], in_=identf[:],
                compare_op=mybir.AluOpType.not_equal, fill=1.0, base=0,
                pattern=[[-1, 64]], channel_multiplier=1)
            # [64, 8x64] = 8 identities side by side: one transpose of
            # [mean|rstd] against this yields the h-repeated broadcast rows
            iw = singles.tile([64, 8, 64], F32, tag="iw")
            nc.gpsimd.memset(iw[:], 0.0)
            idf_ins = nc.gpsimd.affine_select(
                out=iw[:], in_=iw[:],
                compare_op=mybir.AluOpType.not_equal, fill=1.0, base=0,
                pattern=[[0, 8], [-1, 64]], channel_multiplier=1)
            eps_t = singles.tile([2 * BC, 1], F32, tag="eps")
            nc.vector.memset(eps_t[:], EPS)
            ones1 = singles.tile([33, 128], F32, tag="ones1")
            nc.vector.memset(ones1[:], 1.0)

            # ---- DMA issue (sync HWDGE queue, in consumption order) -----
            # Early activations first so the PE starts promptly, then the
            # weights just-in-time.  One queue avoids cross-queue packet
            # round-robin starving the small early tensors.
            t_mk = singles.tile([128, NKC, 2 * BC], BF16, tag="mk")
            d_mk = nc.sync.dma_start(t_mk[:], mk[:])
            t_flt = singles.tile([128, 8, 5 * BC], BF16, tag="flt")
            d_flt = nc.sync.dma_start(t_flt[:], flt[:])
            t_ga = singles.tile([128, NKC, H], BF16, tag="ga")
            d_ga = nc.sync.dma_start(t_ga[:], ga[:])
            t_gb = singles.tile([128, NKC, H], BF16, tag="gb")
            d_gb = nc.sync.dma_start(t_gb[:], gb[:])
            t_we1 = [singles.tile([128, KT_PER_C, H], FP8, tag=f"we1_{c}",
                                  name=f"t_we1_{c}")
                     for c in range(NWE1C)]
            d_we1 = [nc.sync.dma_start(t_we1[c][:], we1c[c][:])
                     for c in range(NWE1C)]
            t_wp1 = singles.tile([128, 8, H], BF16, tag="wp1")
            d_wp1 = nc.sync.dma_start(t_wp1[:], wp1[:])
            t_w2 = singles.tile([128, 24, HH], BF16, tag="w2")
            d_w2 = nc.sync.dma_start(t_w2[:], w2[:])
            t_wc = singles.tile([128, 4, 4], BF16, tag="wc")
            d_wc = nc.sync.dma_start(t_wc[:], wc[:])

            # ---- scalar HWDGE queue: LN/bias params (needed late) -------
            t_p64 = singles.tile([2 * BC, H], F32, tag="p64")
            d_p64 = nc.scalar.dma_start(t_p64[:], p64[:])
            t_gbt = singles.tile([128, 8, 2, 2 * BC], BF16, tag="gbt")
            d_gbt = nc.scalar.dma_start(t_gbt[:], gbt[:])
            t_p32 = singles.tile([BC, 2 * HH + LH + NOUT], F32, tag="p32")
            d_p32 = nc.scalar.dma_start(t_p32[:], p32[:])

            # ---- dep helpers: engine drains absorb multi-waits ----------
            def _raw(inst):
                return inst.ins if hasattr(inst, "ins") else inst

            def engine_absorb(eng, *dep_insts):
                deps = [d for d in dep_insts if d is not None]
                dr = None
                for d in deps:
                    dr = eng.drain(fusable=False)
                    add_dep_helper(_raw(dr), _raw(d), sync=True,
                                   reason="engine observes producer")
                return dr

            # ---- span mean features -> ps_mean [64, 1024] ---------------
            engine_absorb(nc.tensor, d_mk, d_ga, d_gb, d_flt)
            ps_mean = psp.tile([2 * BC, H], F32, tag="ps", name="ps_mean")
            for kc in range(NKC):
                for hf in range(2):
                    nc.tensor.matmul(
                        ps_mean[0:BC, hf * 512:(hf + 1) * 512],
                        lhsT=t_mk[:, kc, 0:BC],
                        rhs=t_ga[:, kc, hf * 512:(hf + 1) * 512],
                        start=(kc == 0), stop=(kc == NKC - 1),
                        skip_group_check=True)
            mean_last = None
            for kc in range(NKC):
                for hf in range(2):
                    mean_last = nc.tensor.matmul(
                        ps_mean[BC:2 * BC, hf * 512:(hf + 1) * 512],
                        lhsT=t_mk[:, kc, BC:2 * BC],
                        rhs=t_gb[:, kc, hf * 512:(hf + 1) * 512],
                        start=(kc == 0), stop=(kc == NKC - 1),
                        skip_group_check=True)

            # means -> sbuf bf16, then transpose to [128, 8, 64]
            pm = singles.tile([2 * BC, H], BF16, tag="pm")
            nc.vector.tensor_copy(pm[:], ps_mean[:])
            pt_span = psp.tile([128, 8, 2 * BC], BF16, tag="ps", name="pt_span")
            span_tr_last = None
            for h in range(8):
                span_tr_last = nc.tensor.transpose(
                    pt_span[:, h, :], pm[:, h * 128:(h + 1) * 128], ident[:])
            mt = singles.tile([128, 8, 2 * BC], BF16, tag="mt")
            mt_cp = nc.vector.tensor_copy(mt[:], pt_span[:])

            # ---- layer 1 into one [64, 1024] psum -----------------------
            ps1 = psp.tile([2 * BC, H], F32, tag="ps", name="ps1")

            # ent branch: rows 32..63; K order: fA, lA, mA, fB, lB, mB
            def ent_lhsT(k):
                f, h = divmod(k, 8)
                if f == 2:
                    return mt[:, h, 0:BC]
                if f == 5:
                    return mt[:, h, BC:2 * BC]
                col = {0: 1, 1: 2, 3: 3, 4: 4}[f] * BC
                return t_flt[:, h, col:col + BC]

            l1e_mm = None
            for k in range(48):
                c, kk = divmod(k, KT_PER_C)
                for hf in range(2):
                    l1e_mm = nc.tensor.matmul(
                        ps1[BC:2 * BC, hf * 512:(hf + 1) * 512],
                        lhsT=ent_lhsT(k),
                        rhs=t_we1[c][:, kk, hf * 512:(hf + 1) * 512],
                        start=(k == 0), stop=(k == 47), skip_group_check=True)

            # pron branch: rows 0..31 (wp1 streams in behind We1)
            engine_absorb(nc.tensor, d_wp1)
            for k in range(8):
                for hf in range(2):
                    nc.tensor.matmul(
                        ps1[0:BC, hf * 512:(hf + 1) * 512],
                        lhsT=t_flt[:, k, 0:BC],
                        rhs=t_wp1[:, k, hf * 512:(hf + 1) * 512],
                        start=(k == 0), stop=(k == 7), skip_group_check=True)

            # junk transposes keep the PE clock (HAM) high while the DVE
            # runs the LN stats chain; they write a scratch psum tile
            junk = psp.tile([64, 64], BF16, tag="ps", name="junk")

            def pe_keepwarm(n):
                for _ in range(n):
                    nc.tensor.matmul(
                        junk[:], lhsT=ident[:], rhs=ident[:],
                        is_transpose=True, start=True, stop=True,
                        skip_group_check=True)

            pe_keepwarm(12)

            # ---- LayerNorm: stats batch-major, rest in transposed space -
            engine_absorb(nc.vector, d_p64, d_gbt, d_p32)
            xsb = singles.tile([2 * BC, H], F32, tag="xsb")
            nc.vector.tensor_add(xsb[:], ps1[:], t_p64[:])
            stats = singles.tile([2 * BC, 2, 6], F32, tag="stats")
            for s in range(2):
                nc.vector.bn_stats(out=stats[:, s, :],
                                   in_=xsb[:, s * 512:(s + 1) * 512])
            mv = singles.tile([2 * BC, 2], F32, tag="mv")
            nc.vector.bn_aggr(out=mv[:], in_=stats[:])
            std = singles.tile([2 * BC, 1], F32, tag="std")
            nc.scalar.activation(
                out=std[:], in_=mv[:, 1:2],
                func=mybir.ActivationFunctionType.Sqrt,
                bias=eps_t[:], scale=1.0)
            # mean in col 0, rstd in col 32 so the transpose lands them on
            # matmul-legal partition bases (0 and 32)
            mv2 = singles.tile([2 * BC, 33], F32, tag="mv2")
            nc.vector.memset(mv2[:], 0.0)
            nc.vector.tensor_copy(mv2[:, 0:1], mv[:, 0:1])
            nc.vector.reciprocal(out=mv2[:, 32:33], in_=std[:])
            engine_absorb(nc.tensor, idf_ins, mean_last)
            pms = psp.tile([33, 8, 2 * BC], F32, tag="ps", name="pms")
            nc.tensor.matmul(pms[:], lhsT=mv2[:], rhs=iw[:],
                             start=True, stop=True, skip_group_check=True)
            ms = singles.tile([33, 8, 2 * BC], F32, tag="ms")
            ms_cp = nc.vector.tensor_copy(ms[0:1, :, :], pms[0:1, :, :])
            ms_cp = nc.vector.tensor_copy(ms[32:33, :, :], pms[32:33, :, :])
            engine_absorb(nc.tensor, ms_cp, span_tr_last)
            pbc = psp.tile([128, 2, 8, 2 * BC], F32, tag="ps", name="pbc")
            for j, base in enumerate((0, 32)):
                nc.tensor.matmul(
                    pbc[:, j, :, :], lhsT=ones1[base:base + 1, :],
                    rhs=ms[base:base + 1, :, :],
                    start=True, stop=True, skip_group_check=True)

            # transpose biased activations -> [128, 8, 64] f32
            pt_x1 = psp.tile([128, 8, 2 * BC], F32, tag="ps", name="pt_x1")
            for h in range(8):
                nc.tensor.transpose(
                    pt_x1[:, h, :], xsb[:, h * 128:(h + 1) * 128], identf[:])
            pe_keepwarm(12)
            xT = singles.tile([128, 8, 2 * BC], F32, tag="xT")
            nc.vector.tensor_copy(xT[:], pt_x1[:])

            nc.vector.tensor_sub(xT[:], xT[:], pbc[:, 0, :, :])
            nc.vector.tensor_mul(xT[:], xT[:], pbc[:, 1, :, :])
            nc.vector.tensor_mul(xT[:], xT[:], t_gbt[:, :, 0, :])
            nc.vector.tensor_add(xT[:], xT[:], t_gbt[:, :, 1, :])
            # leaky: max(x, 0.01x) -> bf16, already transposed for layer 2
            x1t = singles.tile([128, 8, 2 * BC], BF16, tag="x1t")
            nc.vector.scalar_tensor_tensor(
                out=x1t[:], in0=xT[:], scalar=0.01, in1=xT[:],
                op0=mybir.AluOpType.mult, op1=mybir.AluOpType.max)

            # ---- layer 2: [32, 1024] = [xp | xe] ------------------------
            engine_absorb(nc.tensor, d_w2, d_wc)
            ps2 = psp.tile([BC, 2 * HH], F32, tag="ps", name="ps2")
            for k in range(8):
                nc.tensor.matmul(
                    ps2[:, 0:HH], lhsT=x1t[:, k, 0:BC], rhs=t_w2[:, k, :],
                    start=(k == 0), stop=(k == 7), skip_group_check=True)
            for k in range(8):
                nc.tensor.matmul(
                    ps2[:, HH:2 * HH], lhsT=x1t[:, k, BC:2 * BC],
                    rhs=t_w2[:, 8 + k, :],
                    start=(k == 0), stop=(k == 7), skip_group_check=True)
            xcb = singles.tile([BC, 2 * HH], BF16, tag="xcb")
            nc.vector.tensor_add(xcb[:], ps2[:], t_p32[:, 0:2 * HH])

            # transpose xcb -> [128, 8, 32]
            pt_xc = psp.tile([128, 8, BC], BF16, tag="ps", name="pt_xc")
            for h in range(8):
                nc.tensor.transpose(
                    pt_xc[:, h, :], xcb[:, h * 128:(h + 1) * 128],
                    ident[0:BC, 0:BC])
            xct = singles.tile([128, 8, BC], BF16, tag="xct")
            nc.vector.tensor_copy(xct[:], pt_xc[:])

            # ---- layer 3 + exact gelu -----------------------------------
            ps3 = psp.tile([BC, LH], F32, tag="ps", name="ps3")
            for k in range(8):
                nc.tensor.matmul(
                    ps3[:], lhsT=xct[:, k, :], rhs=t_w2[:, 16 + k, :],
                    start=(k == 0), stop=(k == 7), skip_group_check=True)
            g3 = singles.tile([BC, LH], F32, tag="g3")
            nc.vector.tensor_add(g3[:], ps3[:], t_p32[:, 2 * HH:2 * HH + LH])
            geb = singles.tile([BC, LH], BF16, tag="geb")
            nc.scalar.activation(
                out=geb[:], in_=g3[:],
                func=mybir.ActivationFunctionType.Gelu,
                bias=0.0, scale=1.0)

            # transpose -> [128, 4, 32]
            pt_g = psp.tile([128, 4, BC], BF16, tag="ps", name="pt_g")
            for h in range(4):
                nc.tensor.transpose(
                    pt_g[:, h, :], geb[:, h * 128:(h + 1) * 128],
                    ident[0:BC, 0:BC])
            gt = singles.tile([128, 4, BC], BF16, tag="gt")
            nc.vector.tensor_copy(gt[:], pt_g[:])

            # ---- logits -------------------------------------------------
            ps4 = psp.tile([BC, 4], F32, tag="ps", name="ps4")
            for k in range(4):
                nc.tensor.matmul(
                    ps4[:], lhsT=gt[:, k, :], rhs=t_wc[:, k, :],
                    start=(k == 0), stop=(k == 3), skip_group_check=True)
            res = singles.tile([BC, NOUT], F32, tag="res")
            res_add = nc.vector.tensor_add(
                res[:], ps4[:, 0:NOUT],
                t_p32[:, 2 * HH + LH:2 * HH + LH + NOUT])
            engine_absorb(nc.sync, res_add)
            nc.sync.dma_start(out[:], res[:])

    import os
    if not os.environ.get('SKIP_PRUNE'):
        _prune_covered_waits(nc)
    nc.finalize()
    return nc


def _prune_covered_waits(nc):
    """Walrus on this toolchain accepts only one sync-wait on most
    instructions (Drain accepts many).  Within a basic block, same-engine
    instructions execute in order, so a wait already issued by an earlier
    same-engine instruction (e.g. an absorber drain) is redundant on a
    later one and can be dropped.  Any multi-wait instruction left after
    that pruning is split: the extra waits move to prepended single-wait
    Drain instructions on the same engine (all waits must pass before the
    instruction runs, so the order of waiting does not matter)."""
    PRUNABLE = ("DMAHW", "DMASW", "PE_", "DVE_", "Pool_", "Activation_",
                "SP_")

    def prunable(w):
        return (getattr(w, "wait_mode", None) == "sem-ge-imm"
                and w.ant_name.startswith(PRUNABLE))

    for fn in nc.m.functions:
        for blk in fn.blocks:
            observed = {}
            for inst in blk.instructions:
                si = inst.sync_info
                if not si or not si.on_wait:
                    continue
                eng = str(inst.engine)
                kept = []
                for w in si.on_wait:
                    if (prunable(w)
                            and observed.get((eng, w.ant_name), -1)
                            >= w.wait_value):
                        continue
                    kept.append(w)
                for w in si.on_wait:
                    key = (eng, w.ant_name)
                    if prunable(w):
                        if observed.get(key, -1) < w.wait_value:
                            observed[key] = w.wait_value
                if len(kept) != len(si.on_wait):
                    si.on_wait = kept

    for fn in nc.m.functions:
        for blk in fn.blocks:
            insert = []
            for pos, inst in enumerate(blk.instructions):
                si = inst.sync_info
                if si and si.on_wait and len(si.on_wait) > 1:
                    extra = list(si.on_wait[:-1])
                    si.on_wait = [si.on_wait[-1]]
                    insert.append((pos, inst, extra))
            for pos, inst, extra in reversed(insert):
                new_insts = []
                for w in extra:
                    d = mybir.InstDrain(
                        name=nc.get_next_instruction_name(),
                        ins=[], outs=[], bass_is_fusable=False)
                    d.engine = inst.engine
                    d.sync_info = mybir.SyncInfo(on_wait=[w], on_update=[])
                    nc.register_instruction(d)
                    new_insts.append(d)
                blk.instructions[pos:pos] = new_insts


_PROGRAM = None


def _get_program():
    global _PROGRAM
    if _PROGRAM is None:
        _PROGRAM = _build_program()
    return _PROGRAM


_SHARED = None


def _shared_weights(inputs):
    """Per-run shared (batch-independent) weight layouts, computed once."""
    f32 = lambda n: np.ascontiguousarray(np.asarray(inputs[n], np.float32))
    def chunked(a, nk):
        # [nk*128, n] -> [128, nk, n]
        n = a.shape[1]
        return np.ascontiguousarray(
            a.reshape(nk, 128, n).transpose(1, 0, 2).astype(bfloat16))

    Wp1, Wp2, We1, We2, Wl, Wc = (f32(n) for n in
                                  ("Wp1", "Wp2", "We1", "We2", "Wl", "Wc"))
    shared = {"wp1": chunked(Wp1, 8)}
    we1 = (We1 * WE1_SCALE).reshape(48, 128, H).transpose(1, 0, 2)
    we1 = we1.astype(float8_e4m3)
    for c in range(NWE1C):
        shared[f"we1c{c}"] = np.ascontiguousarray(
            we1[:, c * KT_PER_C:(c + 1) * KT_PER_C])
    shared["w2"] = chunked(np.concatenate([Wp2, We2, Wl], axis=0), 24)
    wc = np.zeros((512, 4), np.float32)
    wc[:, :NOUT] = Wc
    shared["wc"] = chunked(wc, 4)

    p64 = np.empty((2 * BC, H), np.float32)
    p64[:BC] = f32("bp1")
    p64[BC:] = f32("be1") * WE1_SCALE   # match the scaled ent psum
    shared["p64"] = p64
    # transposed gamma/beta expanded over the batch dim:
    # gbt[p, h, 0, b] = (gp if b<32 else ge)[h*128+p]; [.., 1, ..] = beta
    g2 = np.stack([f32("gp"), f32("ge")], axis=1)        # [H, 2branch]
    b2_ = np.stack([f32("betap"), f32("betae")], axis=1)
    gb = np.stack([g2, b2_], axis=1)                     # [H, 2gb, 2branch]
    gb = gb.reshape(8, 128, 2, 2).transpose(1, 0, 2, 3)  # [128, 8, 2, 2br]
    shared["gbt"] = np.ascontiguousarray(
        np.repeat(gb, BC, axis=3).astype(bfloat16))      # [128, 8, 2, 64]
    p32 = np.empty((BC, 2 * HH + LH + NOUT), np.float32)
    p32[:, 0:HH] = f32("bp2")
    p32[:, HH:2 * HH] = f32("be2")
    p32[:, 2 * HH:2 * HH + LH] = f32("bl")
    p32[:, 2 * HH + LH:] = f32("bc")
    shared["p32"] = p32
    return shared


def make_in_maps(**inputs):
    """Shard full inputs into per-core input maps (host-side prep)."""
    bert = np.asarray(inputs["bert_outputs"], np.float32)
    offsets = np.asarray(inputs["offsets"], np.int32)
    shared = _shared_weights(inputs)

    in_maps = []
    for c in range(NCORES):
        ob = offsets[c * BC:(c + 1) * BC]
        bc = bert[c * BC:(c + 1) * BC]          # [32, S, H] f32
        m = dict(shared)

        def span_gather(s, e):
            ln = (e - s).astype(np.int64)       # 1..15
            j = np.arange(LSPAN)
            tok = np.minimum(s[:, None] + j[None, :], S - 1)   # [32, 15]
            rows = bc[np.arange(BC)[:, None], tok]             # [32, 15, H]
            g = np.zeros((KPAD, H), np.float32)
            g[:KROWS] = rows.reshape(KROWS, H)
            msk = np.zeros((KPAD, BC), np.float32)
            for b in range(BC):
                msk[b * LSPAN:b * LSPAN + ln[b], b] = 1.0 / ln[b]
            return g, msk

        gA, mskA = span_gather(ob[:, 0], ob[:, 1])
        gB, mskB = span_gather(ob[:, 2], ob[:, 3])
        m["ga"] = np.ascontiguousarray(
            gA.reshape(NKC, 128, H).transpose(1, 0, 2).astype(bfloat16))
        m["gb"] = np.ascontiguousarray(
            gB.reshape(NKC, 128, H).transpose(1, 0, 2).astype(bfloat16))
        msk = np.concatenate([mskA, mskB], axis=1)             # [512, 64]
        m["mk"] = np.ascontiguousarray(
            msk.reshape(NKC, 128, 2 * BC).transpose(1, 0, 2).astype(bfloat16))

        bidx = np.arange(BC)
        rows5 = np.stack([
            bc[bidx, ob[:, 4]],                 # pron
            bc[bidx, ob[:, 0]],                 # firstA
            bc[bidx, ob[:, 1] - 1],             # lastA
            bc[bidx, ob[:, 2]],                 # firstB
            bc[bidx, ob[:, 3] - 1],             # lastB
        ], axis=0)                              # [5, 32, 1024]
        # -> [128, 8, 5*32]: flt[p, h, f*32+b] = rows5[f, b, h*128+p]
        flt = rows5.transpose(2, 0, 1).reshape(8, 128, 5, BC)
        m["flt"] = np.ascontiguousarray(
            flt.transpose(1, 0, 2, 3).reshape(128, 8, 5 * BC).astype(bfloat16))
        in_maps.append(m)
    return in_maps


def run(in_maps, **kwargs):
    nc = _get_program()
    return run_bass_kernel_spmd(nc, in_maps, core_ids=list(range(NCORES)),
                                **kwargs)


def kernel(**inputs):
    res = run(make_in_maps(**inputs))
    return np.concatenate([res.results[c]["out"] for c in range(NCORES)],
                          axis=0).astype(np.float32)


# revision 48
# speedup vs baseline: 1.0099x; 1.0099x over previous
"""Entity-resolution head on 8 TRN2 NeuronCores.

Pure data-parallel: batch dim (256) split 32/core; MLP weights replicated.

Per-core kernel strategy (v2, bf16):
 - Host pregathers the span rows (<=15/span) and the pron/first/last rows,
   casts everything to bf16, and pre-transposes the single-row features so
   the device does no indirect DMA and no transposes for them.
 - Mean span features via mask-stationary matmuls (mask [rows,32] bf16
   stationary, gathered rows [rows,1024] bf16 moving) -> psum [64,1024].
 - MLP matmuls keep activations transposed (features-on-partitions) as the
   stationary operand, weights stream as bf16 moving operand at N=1024.
 - Weights arrive as a few multi-MiB DMAs on the two HWDGE queues
   (sync + scalar), ordered so each layer's weights land just in time.
 - Both branches' layer-1 outputs share one [64,1024] psum so LayerNorm,
   leaky-relu and the affine run once on 64 partitions.
"""

import numpy as np
from ml_dtypes import bfloat16, float8_e4m3

import concourse.bass as bass
import concourse.mybir as mybir
import concourse.tile as tile
from concourse.bass_utils import run_bass_kernel_spmd
from concourse.masks import make_identity
from concourse.tile import add_dep_helper

B, S, H = 256, 512, 1024
HH, LH, NOUT = 512, 512, 3
EPS = 1e-5
NCORES = 8
BC = B // NCORES          # 32 batches per core
LSPAN = 15                # max span length (reference: 1..15)
KROWS = BC * LSPAN        # 480 gathered rows per span side
KPAD = 512                # padded to 4 chunks of 128
NKC = KPAD // 128         # 4 row chunks per side
WE1_SPLITS = [4, 8, 9, 9, 9, 9]      # k-tiles per We1 DMA chunk (sum 48);
NWE1C = len(WE1_SPLITS)              # small first chunk starts l1e early
WE1_OFF = [sum(WE1_SPLITS[:c]) for c in range(NWE1C)]

F32 = mybir.dt.float32
BF16 = mybir.dt.bfloat16
FP8 = mybir.dt.float8e4
WE1_SCALE = 4096.0        # We1 quantized to fp8 at x4096; LN absorbs the
                          # scale (be1 is pre-scaled to match)


def _build_program():
    nc = bass.Bass()

    # ---- DRAM parameters (all host-prepped layouts) --------------------
    ga = nc.declare_dram_parameter("ga", [128, NKC, H], BF16, isOutput=False)
    gb = nc.declare_dram_parameter("gb", [128, NKC, H], BF16, isOutput=False)
    mk = nc.declare_dram_parameter("mk", [128, NKC, 2 * BC], BF16, isOutput=False)
    flt = nc.declare_dram_parameter("flt", [128, 8, 5 * BC], BF16, isOutput=False)
    p64 = nc.declare_dram_parameter("p64", [2 * BC, H], F32, isOutput=False)
    gbt = nc.declare_dram_parameter("gbt", [128, 8, 2, 2 * BC], BF16,
                                    isOutput=False)
    p32 = nc.declare_dram_parameter("p32", [BC, 2 * HH + LH + NOUT], F32,
                                    isOutput=False)
    wp1 = nc.declare_dram_parameter("wp1", [128, 8, H], BF16, isOutput=False)
    we1c = [nc.declare_dram_parameter(f"we1c{c}", [128, WE1_SPLITS[c], H],
                                      FP8, isOutput=False)
            for c in range(NWE1C)]
    w2 = nc.declare_dram_parameter("w2", [128, 24, HH], BF16, isOutput=False)
    wc = nc.declare_dram_parameter("wc", [128, 4, 4], BF16, isOutput=False)
    out = nc.declare_dram_parameter("out", [BC, NOUT], F32, isOutput=True)

    with tile.TileContext(nc) as tc:
        with (
            tc.tile_pool(name="singles", bufs=1) as singles,
            tc.tile_pool(name="ps", bufs=4, space="PSUM") as psp,
        ):
            # ---- small constants -----------------------------------------
            ident = singles.tile([64, 64], BF16, tag="ident")
            make_identity(nc, ident[:])
            identf = singles.tile([64, 64], F32, tag="identf")
            nc.gpsimd.memset(identf[:], 0.0)
            nc.gpsimd.affine_select(
                out=identf[:], in_=identf[:],
                compare_op=mybir.AluOpType.not_equal, fill=1.0, base=0,
                pattern=[[-1, 64]], channel_multiplier=1)
            # [64, 8x64] = 8 identities side by side: one transpose of
            # [mean|rstd] against this yields the h-repeated broadcast rows
            iw = singles.tile([64, 8, 64], F32, tag="iw")
            nc.gpsimd.memset(iw[:], 0.0)
            idf_ins = nc.gpsimd.affine_select(
                out=iw[:], in_=iw[:],
                compare_op=mybir.AluOpType.not_equal, fill=1.0, base=0,
                pattern=[[0, 8], [-1, 64]], channel_multiplier=1)
            eps_t = singles.tile([2 * BC, 1], F32, tag="eps")
            nc.vector.memset(eps_t[:], EPS)
            ones1 = singles.tile([33, 128], F32, tag="ones1")
            nc.vector.memset(ones1[:], 1.0)
            mv2 = singles.tile([2 * BC, 33], F32, tag="mv2")
            nc.vector.memset(mv2[:], 0.0)

            # ---- DMA issue (sync HWDGE queue, in consumption order) -----
            # Early activations first so the PE starts promptly, then the
            # weights just-in-time.  One queue avoids cross-queue packet
            # round-robin starving the small early tensors.
            t_mk = singles.tile([128, NKC, 2 * BC], BF16, tag="mk")
            d_mk = nc.sync.dma_start(t_mk[:], mk[:])
            t_flt = singles.tile([128, 8, 5 * BC], BF16, tag="flt")
            d_flt = nc.sync.dma_start(t_flt[:], flt[:])
            t_ga = singles.tile([128, NKC, H], BF16, tag="ga")
            d_ga = nc.sync.dma_start(t_ga[:], ga[:])
            t_gb = singles.tile([128, NKC, H], BF16, tag="gb")
            d_gb = nc.sync.dma_start(t_gb[:], gb[:])
            t_wp1 = singles.tile([128, 8, H], BF16, tag="wp1")
            d_wp1 = nc.sync.dma_start(t_wp1[:], wp1[:])
            t_we1 = [singles.tile([128, WE1_SPLITS[c], H], FP8,
                                  tag=f"we1_{c}", name=f"t_we1_{c}")
                     for c in range(NWE1C)]
            d_we1 = [nc.sync.dma_start(t_we1[c][:], we1c[c][:])
                     for c in range(NWE1C)]
            t_w2 = singles.tile([128, 24, HH], BF16, tag="w2")
            d_w2 = nc.sync.dma_start(t_w2[:], w2[:])
            t_wc = singles.tile([128, 4, 4], BF16, tag="wc")
            d_wc = nc.sync.dma_start(t_wc[:], wc[:])

            # ---- scalar HWDGE queue: LN/bias params (needed late) -------
            t_p64 = singles.tile([2 * BC, H], F32, tag="p64")
            d_p64 = nc.scalar.dma_start(t_p64[:], p64[:])
            t_gbt = singles.tile([128, 8, 2, 2 * BC], BF16, tag="gbt")
            d_gbt = nc.scalar.dma_start(t_gbt[:], gbt[:])
            t_p32 = singles.tile([BC, 2 * HH + LH + NOUT], F32, tag="p32")
            d_p32 = nc.scalar.dma_start(t_p32[:], p32[:])

            # ---- dep helpers: engine drains absorb multi-waits ----------
            def _raw(inst):
                return inst.ins if hasattr(inst, "ins") else inst

            def engine_absorb(eng, *dep_insts):
                deps = [d for d in dep_insts if d is not None]
                dr = None
                for d in deps:
                    dr = eng.drain(fusable=False)
                    add_dep_helper(_raw(dr), _raw(d), sync=True,
                                   reason="engine observes producer")
                return dr

            # ---- span mean features -> ps_mean [64, 1024] ---------------
            engine_absorb(nc.tensor, d_mk, d_ga, d_gb, d_flt)
            ps_mean = psp.tile([2 * BC, H], F32, tag="ps", name="ps_mean")
            for kc in range(NKC):
                for hf in range(2):
                    nc.tensor.matmul(
                        ps_mean[0:BC, hf * 512:(hf + 1) * 512],
                        lhsT=t_mk[:, kc, 0:BC],
                        rhs=t_ga[:, kc, hf * 512:(hf + 1) * 512],
                        start=(kc == 0), stop=(kc == NKC - 1),
                        skip_group_check=True)
            mean_last = None
            for kc in range(NKC):
                for hf in range(2):
                    mean_last = nc.tensor.matmul(
                        ps_mean[BC:2 * BC, hf * 512:(hf + 1) * 512],
                        lhsT=t_mk[:, kc, BC:2 * BC],
                        rhs=t_gb[:, kc, hf * 512:(hf + 1) * 512],
                        start=(kc == 0), stop=(kc == NKC - 1),
                        skip_group_check=True)

            # means -> sbuf bf16, then transpose to [128, 8, 64]
            pm = singles.tile([2 * BC, H], BF16, tag="pm")
            nc.vector.tensor_copy(pm[:], ps_mean[:])
            pt_span = psp.tile([128, 8, 2 * BC], BF16, tag="ps", name="pt_span")
            span_tr_last = None
            for h in range(8):
                span_tr_last = nc.tensor.transpose(
                    pt_span[:, h, :], pm[:, h * 128:(h + 1) * 128], ident[:])
            mt = singles.tile([128, 8, 2 * BC], BF16, tag="mt")
            mt_cp = nc.vector.tensor_copy(mt[:], pt_span[:])

            # ---- layer 1 into one [64, 1024] psum -----------------------
            ps1 = psp.tile([2 * BC, H], F32, tag="ps", name="ps1")

            # pron branch first: wp1 lands before We1's first chunk, so
            # these fill the PE while the l1e weight stream spins up
            engine_absorb(nc.tensor, d_wp1)
            for k in range(8):
                for hf in range(2):
                    nc.tensor.matmul(
                        ps1[0:BC, hf * 512:(hf + 1) * 512],
                        lhsT=t_flt[:, k, 0:BC],
                        rhs=t_wp1[:, k, hf * 512:(hf + 1) * 512],
                        start=(k == 0), stop=(k == 7), skip_group_check=True)

            # ent branch: rows 32..63; K order: fA, lA, mA, fB, lB, mB
            def ent_lhsT(k):
                f, h = divmod(k, 8)
                if f == 2:
                    return mt[:, h, 0:BC]
                if f == 5:
                    return mt[:, h, BC:2 * BC]
                col = {0: 1, 1: 2, 3: 3, 4: 4}[f] * BC
                return t_flt[:, h, col:col + BC]

            l1e_mm = None
            c = 0
            for k in range(48):
                if c + 1 < NWE1C and k >= WE1_OFF[c + 1]:
                    c += 1
                kk = k - WE1_OFF[c]
                for hf in range(2):
                    l1e_mm = nc.tensor.matmul(
                        ps1[BC:2 * BC, hf * 512:(hf + 1) * 512],
                        lhsT=ent_lhsT(k),
                        rhs=t_we1[c][:, kk, hf * 512:(hf + 1) * 512],
                        start=(k == 0), stop=(k == 47), skip_group_check=True)

            # junk transposes keep the PE clock (HAM) high while the DVE
            # runs the LN chain; they write a scratch psum tile
            junk = psp.tile([64, 64], BF16, tag="ps", name="junk")

            def pe_keepwarm(n):
                for _ in range(n):
                    nc.tensor.matmul(
                        junk[:], lhsT=ident[:], rhs=ident[:],
                        is_transpose=True, start=True, stop=True,
                        skip_group_check=True)

            pe_keepwarm(5)

            # ---- LayerNorm: stats batch-major, rest in transposed space -
            engine_absorb(nc.vector, d_p64, d_gbt, d_p32)
            xsb = singles.tile([2 * BC, H], F32, tag="xsb")
            nc.vector.tensor_add(xsb[:], ps1[:], t_p64[:])

            # transpose biased activations -> [128, 8, 64] f32
            pt_x1 = psp.tile([128, 8, 2 * BC], F32, tag="ps", name="pt_x1")
            for h in range(8):
                nc.tensor.transpose(
                    pt_x1[:, h, :], xsb[:, h * 128:(h + 1) * 128], identf[:])
            pe_keepwarm(10)

            stats = singles.tile([2 * BC, 2, 6], F32, tag="stats")
            for s in range(2):
                nc.vector.bn_stats(out=stats[:, s, :],
                                   in_=xsb[:, s * 512:(s + 1) * 512])
            # mean in col 0, rstd in col 32 so the broadcast matmuls read
            # them from matmul-legal partition bases (0 and 32)
            nc.vector.bn_aggr(out=mv2[:, 0:2], in_=stats[:])
            std = singles.tile([2 * BC, 1], F32, tag="std")
            nc.scalar.activation(
                out=std[:], in_=mv2[:, 1:2],
                func=mybir.ActivationFunctionType.Sqrt,
                bias=eps_t[:], scale=1.0)
            nc.vector.reciprocal(out=mv2[:, 32:33], in_=std[:])
            pms = psp.tile([33, 8, 2 * BC], F32, tag="ps", name="pms")
            nc.tensor.matmul(pms[:], lhsT=mv2[:], rhs=iw[:],
                             start=True, stop=True, skip_group_check=True)
            pe_keepwarm(3)
            ms = singles.tile([33, 8, 2 * BC], F32, tag="ms")
            nc.vector.tensor_copy(ms[0:1, :, :], pms[0:1, :, :])
            nc.vector.tensor_copy(ms[32:33, :, :], pms[32:33, :, :])
            pbc = psp.tile([128, 2, 8, 2 * BC], F32, tag="ps", name="pbc")
            for j, base in enumerate((0, 32)):
                nc.tensor.matmul(
                    pbc[:, j, :, :], lhsT=ones1[base:base + 1, :],
                    rhs=ms[base:base + 1, :, :],
                    start=True, stop=True, skip_group_check=True)
            pe_keepwarm(16)

            xT = singles.tile([128, 8, 2 * BC], F32, tag="xT")
            nc.vector.tensor_copy(xT[:], pt_x1[:])
            nc.vector.tensor_sub(xT[:], xT[:], pbc[:, 0, :, :])
            nc.vector.tensor_mul(xT[:], xT[:], pbc[:, 1, :, :])
            nc.vector.tensor_mul(xT[:], xT[:], t_gbt[:, :, 0, :])
            nc.vector.tensor_add(xT[:], xT[:], t_gbt[:, :, 1, :])
            # leaky: max(x, 0.01x) -> bf16, already transposed for layer 2
            x1t = singles.tile([128, 8, 2 * BC], BF16, tag="x1t")
            nc.vector.scalar_tensor_tensor(
                out=x1t[:], in0=xT[:], scalar=0.01, in1=xT[:],
                op0=mybir.AluOpType.mult, op1=mybir.AluOpType.max)

            # ---- layer 2: [32, 1024] = [xp | xe] ------------------------
            engine_absorb(nc.tensor, d_w2, d_wc)
            ps2 = psp.tile([BC, 2 * HH], F32, tag="ps", name="ps2")
            for k in range(8):
                nc.tensor.matmul(
                    ps2[:, 0:HH], lhsT=x1t[:, k, 0:BC], rhs=t_w2[:, k, :],
                    start=(k == 0), stop=(k == 7), skip_group_check=True)
            for k in range(8):
                nc.tensor.matmul(
                    ps2[:, HH:2 * HH], lhsT=x1t[:, k, BC:2 * BC],
                    rhs=t_w2[:, 8 + k, :],
                    start=(k == 0), stop=(k == 7), skip_group_check=True)
            xcb = singles.tile([BC, 2 * HH], BF16, tag="xcb")
            nc.vector.tensor_add(xcb[:], ps2[:], t_p32[:, 0:2 * HH])

            # transpose xcb -> [128, 8, 32]
            pt_xc = psp.tile([128, 8, BC], BF16, tag="ps", name="pt_xc")
            for h in range(8):
                nc.tensor.transpose(
                    pt_xc[:, h, :], xcb[:, h * 128:(h + 1) * 128],
                    ident[0:BC, 0:BC])
            xct = singles.tile([128, 8, BC], BF16, tag="xct")
            nc.vector.tensor_copy(xct[:], pt_xc[:])

            # ---- layer 3 + exact gelu -----------------------------------
            ps3 = psp.tile([BC, LH], F32, tag="ps", name="ps3")
            for k in range(8):
                nc.tensor.matmul(
                    ps3[:], lhsT=xct[:, k, :], rhs=t_w2[:, 16 + k, :],
                    start=(k == 0), stop=(k == 7), skip_group_check=True)
            g3 = singles.tile([BC, LH], F32, tag="g3")
            nc.vector.tensor_add(g3[:], ps3[:], t_p32[:, 2 * HH:2 * HH + LH])
            geb = singles.tile([BC, LH], BF16, tag="geb")
            nc.scalar.activation(
                out=geb[:], in_=g3[:],
                func=mybir.ActivationFunctionType.Gelu,
                bias=0.0, scale=1.0)

            # transpose -> [128, 4, 32]
            pt_g = psp.tile([128, 4, BC], BF16, tag="ps", name="pt_g")
            for h in range(4):
                nc.tensor.transpose(
                    pt_g[:, h, :], geb[:, h * 128:(h + 1) * 128],
                    ident[0:BC, 0:BC])
            gt = singles.tile([128, 4, BC], BF16, tag="gt")
            nc.vector.tensor_copy(gt[:], pt_g[:])

            # ---- logits -------------------------------------------------
            ps4 = psp.tile([BC, 4], F32, tag="ps", name="ps4")
            for k in range(4):
                nc.tensor.matmul(
                    ps4[:], lhsT=gt[:, k, :], rhs=t_wc[:, k, :],
                    start=(k == 0), stop=(k == 3), skip_group_check=True)
            res = singles.tile([BC, NOUT], F32, tag="res")
            res_add = nc.vector.tensor_add(
                res[:], ps4[:, 0:NOUT],
                t_p32[:, 2 * HH + LH:2 * HH + LH + NOUT])
            engine_absorb(nc.sync, res_add)
            nc.sync.dma_start(out[:], res[:])

    import os
    if not os.environ.get('SKIP_PRUNE'):
        _prune_covered_waits(nc)
    nc.finalize()
    return nc


def _prune_covered_waits(nc):
    """Walrus on this toolchain accepts only one sync-wait on most
    instructions (Drain accepts many).  Within a basic block, same-engine
    instructions execute in order, so a wait already issued by an earlier
    same-engine instruction (e.g. an absorber drain) is redundant on a
    later one and can be dropped.  Any multi-wait instruction left after
    that pruning is split: the extra waits move to prepended single-wait
    Drain instructions on the same engine (all waits must pass before the
    instruction runs, so the order of waiting does not matter)."""
    PRUNABLE = ("DMAHW", "DMASW", "PE_", "DVE_", "Pool_", "Activation_",
                "SP_")

    def prunable(w):
        return (getattr(w, "wait_mode", None) == "sem-ge-imm"
                and w.ant_name.startswith(PRUNABLE))

    for fn in nc.m.functions:
        for blk in fn.blocks:
            observed = {}
            for inst in blk.instructions:
                si = inst.sync_info
                if not si or not si.on_wait:
                    continue
                eng = str(inst.engine)
                kept = []
                for w in si.on_wait:
                    if (prunable(w)
                            and observed.get((eng, w.ant_name), -1)
                            >= w.wait_value):
                        continue
                    kept.append(w)
                for w in si.on_wait:
                    key = (eng, w.ant_name)
                    if prunable(w):
                        if observed.get(key, -1) < w.wait_value:
                            observed[key] = w.wait_value
                if len(kept) != len(si.on_wait):
                    si.on_wait = kept

    for fn in nc.m.functions:
        for blk in fn.blocks:
            insert = []
            for pos, inst in enumerate(blk.instructions):
                si = inst.sync_info
                if si and si.on_wait and len(si.on_wait) > 1:
                    extra = list(si.on_wait[:-1])
                    si.on_wait = [si.on_wait[-1]]
                    insert.append((pos, inst, extra))
            for pos, inst, extra in reversed(insert):
                new_insts = []
                for w in extra:
                    d = mybir.InstDrain(
                        name=nc.get_next_instruction_name(),
                        ins=[], outs=[], bass_is_fusable=False)
                    d.engine = inst.engine
                    d.sync_info = mybir.SyncInfo(on_wait=[w], on_update=[])
                    nc.register_instruction(d)
                    new_insts.append(d)
                blk.instructions[pos:pos] = new_insts


_PROGRAM = None


def _get_program():
    global _PROGRAM
    if _PROGRAM is None:
        _PROGRAM = _build_program()
    return _PROGRAM


_SHARED = None


def _shared_weights(inputs):
    """Per-run shared (batch-independent) weight layouts, computed once."""
    f32 = lambda n: np.ascontiguousarray(np.asarray(inputs[n], np.float32))
    def chunked(a, nk):
        # [nk*128, n] -> [128, nk, n]
        n = a.shape[1]
        return np.ascontiguousarray(
            a.reshape(nk, 128, n).transpose(1, 0, 2).astype(bfloat16))

    Wp1, Wp2, We1, We2, Wl, Wc = (f32(n) for n in
                                  ("Wp1", "Wp2", "We1", "We2", "Wl", "Wc"))
    shared = {"wp1": chunked(Wp1, 8)}
    we1 = (We1 * WE1_SCALE).reshape(48, 128, H).transpose(1, 0, 2)
    we1 = we1.astype(float8_e4m3)
    for c in range(NWE1C):
        shared[f"we1c{c}"] = np.ascontiguousarray(
            we1[:, WE1_OFF[c]:WE1_OFF[c] + WE1_SPLITS[c]])
    shared["w2"] = chunked(np.concatenate([Wp2, We2, Wl], axis=0), 24)
    wc = np.zeros((512, 4), np.float32)
    wc[:, :NOUT] = Wc
    shared["wc"] = chunked(wc, 4)

    p64 = np.empty((2 * BC, H), np.float32)
    p64[:BC] = f32("bp1")
    p64[BC:] = f32("be1") * WE1_SCALE   # match the scaled ent psum
    shared["p64"] = p64
    # transposed gamma/beta expanded over the batch dim:
    # gbt[p, h, 0, b] = (gp if b<32 else ge)[h*128+p]; [.., 1, ..] = beta
    g2 = np.stack([f32("gp"), f32("ge")], axis=1)        # [H, 2branch]
    b2_ = np.stack([f32("betap"), f32("betae")], axis=1)
    gb = np.stack([g2, b2_], axis=1)                     # [H, 2gb, 2branch]
    gb = gb.reshape(8, 128, 2, 2).transpose(1, 0, 2, 3)  # [128, 8, 2, 2br]
    shared["gbt"] = np.ascontiguousarray(
        np.repeat(gb, BC, axis=3).astype(bfloat16))      # [128, 8, 2, 64]
    p32 = np.empty((BC, 2 * HH + LH + NOUT), np.float32)
    p32[:, 0:HH] = f32("bp2")
    p32[:, HH:2 * HH] = f32("be2")
    p32[:, 2 * HH:2 * HH + LH] = f32("bl")
    p32[:, 2 * HH + LH:] = f32("bc")
    shared["p32"] = p32
    return shared


def make_in_maps(**inputs):
    """Shard full inputs into per-core input maps (host-side prep)."""
    bert = np.asarray(inputs["bert_outputs"], np.float32)
    offsets = np.asarray(inputs["offsets"], np.int32)
    shared = _shared_weights(inputs)

    in_maps = []
    for c in range(NCORES):
        ob = offsets[c * BC:(c + 1) * BC]
        bc = bert[c * BC:(c + 1) * BC]          # [32, S, H] f32
        m = dict(shared)

        def span_gather(s, e):
            ln = (e - s).astype(np.int64)       # 1..15
            j = np.arange(LSPAN)
            tok = np.minimum(s[:, None] + j[None, :], S - 1)   # [32, 15]
            rows = bc[np.arange(BC)[:, None], tok]             # [32, 15, H]
            g = np.zeros((KPAD, H), np.float32)
            g[:KROWS] = rows.reshape(KROWS, H)
            msk = np.zeros((KPAD, BC), np.float32)
            for b in range(BC):
                msk[b * LSPAN:b * LSPAN + ln[b], b] = 1.0 / ln[b]
            return g, msk

        gA, mskA = span_gather(ob[:, 0], ob[:, 1])
        gB, mskB = span_gather(ob[:, 2], ob[:, 3])
        m["ga"] = np.ascontiguousarray(
            gA.reshape(NKC, 128, H).transpose(1, 0, 2).astype(bfloat16))
        m["gb"] = np.ascontiguousarray(
            gB.reshape(NKC, 128, H).transpose(1, 0, 2).astype(bfloat16))
        msk = np.concatenate([mskA, mskB], axis=1)             # [512, 64]
        m["mk"] = np.ascontiguousarray(
            msk.reshape(NKC, 128, 2 * BC).transpose(1, 0, 2).astype(bfloat16))

        bidx = np.arange(BC)
        rows5 = np.stack([
            bc[bidx, ob[:, 4]],                 # pron
            bc[bidx, ob[:, 0]],                 # firstA
            bc[bidx, ob[:, 1] - 1],             # lastA
            bc[bidx, ob[:, 2]],                 # firstB
            bc[bidx, ob[:, 3] - 1],             # lastB
        ], axis=0)                              # [5, 32, 1024]
        # -> [128, 8, 5*32]: flt[p, h, f*32+b] = rows5[f, b, h*128+p]
        flt = rows5.transpose(2, 0, 1).reshape(8, 128, 5, BC)
        m["flt"] = np.ascontiguousarray(
            flt.transpose(1, 0, 2, 3).reshape(128, 8, 5 * BC).astype(bfloat16))
        in_maps.append(m)
    return in_maps


def run(in_maps, **kwargs):
    nc = _get_program()
    return run_bass_kernel_spmd(nc, in_maps, core_ids=list(range(NCORES)),
                                **kwargs)


def kernel(**inputs):
    res = run(make_in_maps(**inputs))
    return np.concatenate([res.results[c]["out"] for c in range(NCORES)],
                          axis=0).astype(np.float32)
